# revision 28
# baseline (speedup 1.0000x reference)
"""GAT (2-layer, PyG-style) on 8 Trainium2 NeuronCores.

Strategy (node/graph-parallel, dst-sharded edges):
  - Nodes partitioned into 8 contiguous ranges (6250/core); edges assigned to
    the core owning their DST node, sorted by dst, grouped per 128-dst window
    and split by table-row parity (see below), padded to 128-edge tiles.
  - Node phase sharded: each core computes its rows of the gather table
    (512-B rows: 256 fp8 features [+bias], then bf16 a_src/a_dst coefs),
    then a chunked AllGather replicates the table to every core's DRAM.
  - Edge phase per core: per-edge source rows fetched with TWO batched
    dma_gather instructions per dst-window (the int16-index SWDGE gather;
    parity split with a 1024-B stride makes 50000 rows addressable with
    int16 indices).  Attention logits assembled on-chip; segment softmax +
    scatter-add are TensorE matmuls against 0/1 onehot matrices uploaded as
    u8 (shared by both layers) and cast on-chip; exp folded into the matmul
    rhs, 1/sum applied per-dst at the end.  leaky_relu/exp run on ScalarE.
  - Global mean-pool via matmul with a host-built node->graph 0/1 map,
    AllReduce of the [256, 50] partial, then the FC layer replicated.
"""

import os
import sys

sys.path.insert(0, "/opt/trn_rl_repo")

import numpy as np
import ml_dtypes

N_NODES, N_EDGES = 50000, 800000
IN_C, HID_C, OUT_C, HEADS = 256, 64, 256, 4
N_GRAPHS = 50
NEG_SLOPE = 0.2
NCORES = 8
WIN = 128         # dst nodes per aggregation window (psum partitions)
ROWB = 256        # table row width in bf16 elems (512 bytes)
P = 128

BF16 = ml_dtypes.bfloat16

LAST_EXEC_NS = None  # set by kernel() when GAT_TRACE=1


# --------------------------------------------------------------------------
# host-side preprocessing
# --------------------------------------------------------------------------

def build_edge_data(src, dst, rowmap, n_nodes, ncores, win):
    """Per-core int16 gather indices + u8 onehot matrices.

    Edges are dst-sorted, grouped per (core, window), split by parity of
    their source's table row (dma_gather int16 indices address rows at a
    1024-B stride: even rows via base 0, odd rows via base +512 B), and
    padded to 128-edge tiles per parity.

    Returns (TE, TO, woff, percore): TE[w]/TO[w] = even/odd tile count of
    window w (same for all cores); percore[c] = dict(
      i16 [128, Ttot*8] i16, ohe [128, Ttot, 128] u8, ohd [128, Ttot, 128] u8).
    """
    nc_nodes = n_nodes // ncores
    nwin = (nc_nodes + win - 1) // win
    E = src.shape[0]
    r = rowmap[src]                      # table row of each edge's source
    par = (r & 1).astype(np.int64)
    ridx = (r >> 1).astype(np.int16)

    core_of = dst // nc_nodes
    win_of = (dst % nc_nodes) // win
    dloc = (dst - (core_of * nc_nodes + win_of * win)).astype(np.int64)

    # group edges by (core, window, parity) — groups must be contiguous for
    # the rank-in-group slot assignment below
    order = np.lexsort((dst, par, win_of, core_of))
    s_core, s_win, s_par = core_of[order], win_of[order], par[order]
    s_ridx, s_dloc = ridx[order], dloc[order]

    gid = (s_core * nwin + s_win) * 2 + s_par
    sizes = np.bincount(gid, minlength=ncores * nwin * 2)
    starts = np.concatenate([[0], np.cumsum(sizes)])[:-1]
    k_in_g = np.arange(E) - starts[gid]

    cnt = sizes.reshape(ncores, nwin, 2)
    TE = np.maximum((cnt[:, :, 0].max(axis=0) + P - 1) // P, 1).astype(np.int64)
    TO = np.maximum((cnt[:, :, 1].max(axis=0) + P - 1) // P, 1).astype(np.int64)
    # gather index counts: max real count across cores rounded to 16 (the
    # output still covers TE/TO full tiles; trailing slots are never written
    # and rely on pre-zeroed gather buffers)
    NE = np.clip(((cnt[:, :, 0].max(axis=0) + 15) // 16) * 16, 16, TE * P).astype(np.int64)
    NO = np.clip(((cnt[:, :, 1].max(axis=0) + 15) // 16) * 16, 16, TO * P).astype(np.int64)
    twin = TE + TO
    woff = np.concatenate([[0], np.cumsum(twin)])[:-1]
    ttot = int(twin.sum())

    base_tile = woff[s_win] + np.where(s_par == 1, TE[s_win], 0)
    gtile = base_tile + k_in_g // P
    lane = (k_in_g % P).astype(np.int64)
    # int16 index array position: block col base + k//16, row k%16
    col = base_tile * 8 + k_in_g // 16
    row16 = (k_in_g % 16).astype(np.int64)

    percore = []
    for c in range(ncores):
        m = s_core == c
        i16 = np.zeros((16, ttot * 8), dtype=np.int16)
        i16[row16[m], col[m]] = s_ridx[m]
        ohe = np.zeros((P, ttot, P), dtype=np.uint8)
        ohd = np.zeros((P, ttot, P), dtype=np.uint8)
        ohe[lane[m], gtile[m], s_dloc[m]] = 1
        ohd[s_dloc[m], gtile[m], lane[m]] = 1
        percore.append(dict(i16=np.tile(i16, (8, 1)), ohe=ohe, ohd=ohd))
    return TE, TO, NE, NO, woff, percore


def balance_nodes(dst, n_nodes, ncores, win):
    """Relabel nodes so each (core, window) bin carries a near-equal edge
    count: perm[old_id] = new_id. Greedy largest-degree-first into the
    lightest non-full bin."""
    import heapq
    deg = np.bincount(dst, minlength=n_nodes).astype(np.int64)
    nc_nodes = n_nodes // ncores
    nwin = (nc_nodes + win - 1) // win
    base = []
    cap = []
    for c in range(ncores):
        for w in range(nwin):
            base.append(c * nc_nodes + w * win)
            cap.append(min(win, nc_nodes - w * win))
    nbins = len(base)
    order = np.argsort(-deg, kind="stable")
    heap = [(0, b) for b in range(nbins)]
    heapq.heapify(heap)
    slot = [0] * nbins
    perm = np.zeros(n_nodes, dtype=np.int64)
    for node in order:
        while True:
            load, b = heapq.heappop(heap)
            if slot[b] < cap[b]:
                break
        perm[node] = base[b] + slot[b]
        slot[b] += 1
        if slot[b] < cap[b]:
            heapq.heappush(heap, (load + deg[node], b))
    return perm

CHUNK_TILES = 7  # node-tiles per AllGather chunk


def chunk_layout(n_nodes, ncores, chunk_tiles):
    """Chunked-AllGather table layout. Returns (bounds, rowmap) where bounds
    are per-core local row boundaries of each chunk and rowmap[node] is the
    table row of a global node id under chunk-major ordering."""
    nc_nodes = n_nodes // ncores
    bounds = []
    lo = 0
    while lo < nc_nodes:
        hi = min(lo + chunk_tiles * P, nc_nodes)
        bounds.append((lo, hi))
        lo = hi
    rowmap = np.zeros(n_nodes, dtype=np.int64)
    out_base = 0
    for (lo, hi) in bounds:
        s = hi - lo
        for c in range(ncores):
            nodes = np.arange(c * nc_nodes + lo, c * nc_nodes + hi)
            rowmap[nodes] = out_base + c * s + np.arange(s)
        out_base += ncores * s
    return bounds, rowmap


def build_host_inputs(x, edge_index, batch, W1, att_src1, att_dst1, b1,
                      W2, att_src2, att_dst2, b2, Wfc, bfc,
                      n_nodes, n_graphs, ncores, win):
    src, dst = np.asarray(edge_index[0]), np.asarray(edge_index[1])
    nc_nodes = n_nodes // ncores
    nt = (nc_nodes + P - 1) // P
    ncpad = nt * P

    bounds, rowmap = chunk_layout(n_nodes, ncores, CHUNK_TILES)
    TE, TO, NE, NO, woff, edata = build_edge_data(
        src.astype(np.int64), dst.astype(np.int64), rowmap,
        n_nodes, ncores, win)

    # augmented weights: a = x @ (W @ att) computed in the same matmul as h
    in_c = W1.shape[0]
    A1 = np.zeros((in_c, 2 * HEADS), dtype=np.float64)
    for h in range(HEADS):
        A1[:, h] = W1[:, h * HID_C:(h + 1) * HID_C].astype(np.float64) @ att_src1[h].astype(np.float64)
        A1[:, HEADS + h] = W1[:, h * HID_C:(h + 1) * HID_C].astype(np.float64) @ att_dst1[h].astype(np.float64)
    W1aug = np.concatenate([W1.astype(np.float64), A1], axis=1).astype(BF16)  # [in_c, 264]

    hid2 = W2.shape[0]
    A2 = np.zeros((hid2, 2), dtype=np.float64)
    A2[:, 0] = W2.astype(np.float64) @ att_src2[0].astype(np.float64)
    A2[:, 1] = W2.astype(np.float64) @ att_dst2[0].astype(np.float64)
    W2aug = np.concatenate([W2.astype(np.float64), A2], axis=1).astype(BF16)  # [hid2, 258]

    # graph-mean map and counts
    cnt = np.bincount(batch, minlength=n_graphs).astype(np.float32)
    cnt_inv = (1.0 / np.maximum(cnt, 1.0)).astype(np.float32)

    out_c = Wfc.shape[0]
    has_b1 = bool(np.any(b1))
    has_b2 = bool(np.any(b2))
    common = dict(
        w1aug=np.ascontiguousarray(W1aug),
        w2aug=np.ascontiguousarray(W2aug),
        wfc=np.ascontiguousarray(Wfc.astype(BF16)),
        bfc2=np.ascontiguousarray(bfc.astype(np.float32).reshape(2, P).T.copy()),
        cinv=np.ascontiguousarray(np.broadcast_to(cnt_inv, (P, n_graphs)).copy()),
    )
    if has_b1:
        common["b1rep"] = np.ascontiguousarray(
            np.broadcast_to(b1.astype(np.float32), (P, b1.shape[0])).copy())
    if has_b2:
        common["b2rep"] = np.ascontiguousarray(
            np.broadcast_to(b2.astype(np.float32), (P, b2.shape[0])).copy())

    per_core = []
    for c in range(ncores):
        xt = np.zeros((in_c, ncpad), dtype=BF16)
        xs = x[c * nc_nodes:(c + 1) * nc_nodes].astype(np.float32)
        xt[:, :nc_nodes] = np.ascontiguousarray(xs.T).astype(BF16)
        gmap = np.zeros((nt, P, n_graphs), dtype=np.float32)
        nodes = np.arange(nc_nodes)
        gmap[nodes // P, nodes % P, batch[c * nc_nodes:(c + 1) * nc_nodes]] = 1.0
        d = edata[c]
        per_core.append(dict(
            xt=xt,
            i16=np.ascontiguousarray(d["i16"]),
            ohe=np.ascontiguousarray(d["ohe"]),
            ohd=np.ascontiguousarray(d["ohd"]),
            gmap=np.ascontiguousarray(gmap.astype(BF16)),
            **common,
        ))
    return TE, TO, NE, NO, woff, has_b1, has_b2, per_core


# --------------------------------------------------------------------------
# device program
# --------------------------------------------------------------------------

def build_program(TE, TO, NE, NO, woff, has_b1, has_b2, n_nodes, n_graphs, ncores, win,
                  dma_scratch=49152):
    bounds, _ = chunk_layout(n_nodes, ncores, CHUNK_TILES)
    from concourse import bass, bacc, mybir, tile
    from concourse.masks import make_identity

    DT = mybir.dt.bfloat16
    F8 = mybir.dt.float8e4
    U8 = mybir.dt.uint8
    F32 = mybir.dt.float32
    AF = mybir.ActivationFunctionType
    OP = mybir.AluOpType

    nc_nodes = n_nodes // ncores
    nt = (nc_nodes + P - 1) // P
    nwin = (nc_nodes + win - 1) // win
    twin = TE + TO
    ttot = int(np.sum(twin))
    tmax = int(np.max(twin))
    in_c, out_c = IN_C, OUT_C
    G = n_graphs

    nc = bacc.Bacc("TRN2", target_bir_lowering=False, num_devices=ncores,
                   dynamic_dma_scratch_size=dma_scratch)

    # ---- dram i/o ----
    xt_d = nc.dram_tensor("xt", [in_c, nt * P], DT, kind="ExternalInput")
    w1_d = nc.dram_tensor("w1aug", [in_c, 264], DT, kind="ExternalInput")
    w2_d = nc.dram_tensor("w2aug", [in_c, 258], DT, kind="ExternalInput")
    wfc_d = nc.dram_tensor("wfc", [out_c, out_c], DT, kind="ExternalInput")
    i16_d = nc.dram_tensor("i16", [P, ttot * 8], mybir.dt.int16, kind="ExternalInput")
    ohe_d = nc.dram_tensor("ohe", [P, ttot, P], U8, kind="ExternalInput")
    ohd_d = nc.dram_tensor("ohd", [P, ttot, P], U8, kind="ExternalInput")
    gmap_d = nc.dram_tensor("gmap", [nt, P, G], DT, kind="ExternalInput")
    bfc_d = nc.dram_tensor("bfc2", [P, 2], F32, kind="ExternalInput")
    cinv_d = nc.dram_tensor("cinv", [P, G], F32, kind="ExternalInput")
    b1_d = nc.dram_tensor("b1rep", [P, out_c], F32, kind="ExternalInput") if has_b1 else None
    b2_d = nc.dram_tensor("b2rep", [P, out_c], F32, kind="ExternalInput") if has_b2 else None
    y_d = nc.dram_tensor("y", [out_c, G], F32, kind="ExternalOutput")

    cin1 = nc.dram_tensor("cin1", [nc_nodes, ROWB], DT, kind="Internal")
    tab1 = nc.dram_tensor("tab1", [n_nodes, ROWB], DT, kind="Internal", addr_space="Shared")
    cin2 = nc.dram_tensor("cin2", [nc_nodes, ROWB], DT, kind="Internal")
    tab2 = nc.dram_tensor("tab2", [n_nodes, ROWB], DT, kind="Internal", addr_space="Shared")
    pin = nc.dram_tensor("pin", [out_c, G], F32, kind="Internal")
    pout = nc.dram_tensor("pout", [out_c, G], F32, kind="Internal", addr_space="Shared")

    groups = [list(range(ncores))]

    with tile.TileContext(nc) as tc:
        with (
            tc.tile_pool(name="const", bufs=1) as cpool,
            tc.tile_pool(name="work", bufs=3) as wpool,
            tc.tile_pool(name="gath", bufs=3) as gpool,
            tc.tile_pool(name="np", bufs=2, space="PSUM") as npp,
            tc.tile_pool(name="agg", bufs=2, space="PSUM") as aggp,
            tc.tile_pool(name="adp", bufs=1, space="PSUM") as adp,
            tc.tile_pool(name="trp", bufs=1, space="PSUM") as trp,
            tc.tile_pool(name="plp", bufs=1, space="PSUM") as plp,
        ):
            # ---- constants ----
            ident = cpool.tile([P, P], DT)
            make_identity(nc, ident[:])
            w1_sb = cpool.tile([P, 2, 264], DT)
            nc.sync.dma_start(out=w1_sb[:, :, :], in_=w1_d.ap().rearrange("(kh p) m -> p kh m", p=P))
            w2_sb = cpool.tile([P, 2, 258], DT)
            nc.sync.dma_start(out=w2_sb[:, :, :], in_=w2_d.ap().rearrange("(kh p) m -> p kh m", p=P))
            wfc_sb = cpool.tile([P, 2, 2, P], DT)  # [k-half, m-half]
            nc.sync.dma_start(out=wfc_sb[:, :, :, :],
                              in_=wfc_d.ap().rearrange("(kh p) (mh q) -> p kh mh q", p=P, q=P))
            bfc_sb = cpool.tile([P, 2], F32)
            nc.sync.dma_start(out=bfc_sb[:, :], in_=bfc_d[:, :])
            cinv_sb = cpool.tile([P, G], F32)
            nc.sync.dma_start(out=cinv_sb[:, :], in_=cinv_d[:, :])
            i16_sb = cpool.tile([P, ttot * 8], mybir.dt.int16)
            nc.sync.dma_start(out=i16_sb[:, :], in_=i16_d[:, :])
            b1_sb = b2_sb = None
            if has_b1:
                b1_sb = cpool.tile([P, out_c], F32)
                nc.sync.dma_start(out=b1_sb[:, :], in_=b1_d[:, :])
            if has_b2:
                b2_sb = cpool.tile([P, out_c], F32)
                nc.sync.dma_start(out=b2_sb[:, :], in_=b2_d[:, :])

            tab1_3 = tab1.ap().rearrange("(r two) c -> r two c", two=2)
            tab2_3 = tab2.ap().rearrange("(r two) c -> r two c", two=2)

            # ---- node phase ----
            chunk_end_tile = {}
            chunk_ob = []
            ob = 0
            for k, (lo, hi) in enumerate(bounds):
                chunk_end_tile[(hi + P - 1) // P - 1] = k
                chunk_ob.append(ob)
                ob += ncores * (hi - lo)

            def ag_chunk(cin, tab, k):
                lo, hi = bounds[k]
                s = hi - lo
                nc.gpsimd.collective_compute(
                    "AllGather", mybir.AluOpType.bypass,
                    ins=[cin.ap()[lo:hi, :]],
                    outs=[tab.ap()[chunk_ob[k]:chunk_ob[k] + ncores * s, :]],
                    replica_groups=groups)

            def node_tile(t, lhsT_of, w_sb, ocols, cin, brep):
                rows = min(P, nc_nodes - t * P)
                na = ocols - 256
                ps = npp.tile([P, 264], F32, tag="nps", name="nps")
                for kh in range(2):
                    nc.tensor.matmul(out=ps[:rows, :ocols], lhsT=lhsT_of(t, kh, rows),
                                     rhs=w_sb[:, kh, :ocols], start=(kh == 0), stop=(kh == 1))
                sb = wpool.tile([P, ROWB], DT, tag="nsb", name="nsb")
                f8v = sb[:rows, :].bitcast(F8)
                if brep is not None:
                    nc.vector.tensor_tensor(out=f8v[:, 0:256], in0=ps[:rows, 0:256],
                                            in1=brep[:rows, :], op=OP.add)
                else:
                    nc.vector.tensor_copy(out=f8v[:, 0:256], in_=ps[:rows, 0:256])
                nc.vector.tensor_copy(out=sb[:rows, 128:128 + na], in_=ps[:rows, 256:ocols])
                nc.vector.memset(sb[:rows, 128 + na:ROWB], 0)
                nc.sync.dma_start(out=cin.ap()[t * P:t * P + rows, :], in_=sb[:rows, :])

            def node_phase(lhsT_of, w_sb, ocols, cin, brep, tab=None):
                for t in range(nt):
                    node_tile(t, lhsT_of, w_sb, ocols, cin, brep)
                    if tab is not None and t in chunk_end_tile:
                        ag_chunk(cin, tab, chunk_end_tile[t])

            # layer-1 node phase: xT staged in bulk DMAs, sliced per tile
            xt_sb = cpool.tile([P, 2, nt * P], DT)
            xstep = ((nt + 4) // 5) * P
            for lo in range(0, nt * P, xstep):
                hi = min(lo + xstep, nt * P)
                for kh in range(2):
                    nc.sync.dma_start(out=xt_sb[:, kh, lo:hi], in_=xt_d[kh * P:(kh + 1) * P, lo:hi])
            def l1_lhsT(t, kh, rows):
                return xt_sb[:, kh, t * P:t * P + rows]
            node_phase(l1_lhsT, w1_sb, 264, cin1, b1_sb, tab=tab1)

            # ---- edge phase ----
            def edge_phase(tab3, cin, H, after_window=None, pool_into=None):
                """Table row bf16-col layout: feats fp8 in bytes [0:256)
                (bf16 cols 0:128), a_src bf16 cols 128:128+H, a_dst cols
                128+H:128+2H."""
                RH = H + out_c
                for w in range(nwin):
                    size = min(win, nc_nodes - w * win)
                    te, to = int(TE[w]), int(TO[w])
                    tw = te + to
                    t0 = int(woff[w])
                    # own-dst attention coefs
                    ad = wpool.tile([win, HEADS], DT, tag="adst")
                    nc.sync.dma_start(out=ad[:size, :H],
                                      in_=cin.ap()[w * win:w * win + size, 128 + H:128 + 2 * H])
                    # onehots: u8 upload (shared by both layers), cast on-chip
                    oheu = wpool.tile([P, tmax, P], U8, tag="oheu")
                    nc.sync.dma_start(out=oheu[:, :tw, :], in_=ohe_d.ap()[:, t0:t0 + tw, :])
                    ohe = wpool.tile([P, tmax, win], DT, tag="ohe")
                    nc.vector.tensor_copy(out=ohe[:, :tw, :], in_=oheu[:, :tw, :])
                    ohdu = wpool.tile([win, tmax, P], U8, tag="ohdu")
                    nc.sync.dma_start(out=ohdu[:, :tw, :], in_=ohd_d.ap()[:, t0:t0 + tw, :])
                    ohd = wpool.tile([win, tmax, P], DT, tag="ohd")
                    nc.vector.tensor_copy(out=ohd[:, :tw, :], in_=ohdu[:, :tw, :])
                    # gather source rows: one batched SWDGE gather per parity
                    g = gpool.tile([P, tmax, ROWB], DT, tag="g")
                    ne, no = int(NE[w]), int(NO[w])
                    if te:
                        nc.gpsimd.dma_gather(
                            out_ap=g[:, 0:te, :], in_ap=tab3[:, 0, :],
                            idxs_ap=i16_sb[:, t0 * 8:t0 * 8 + ne // 16],
                            num_idxs=ne, num_idxs_reg=ne,
                            elem_size=ROWB, elem_step=2 * ROWB,
                            single_packet=False)
                    if to:
                        nc.gpsimd.dma_gather(
                            out_ap=g[:, te:tw, :], in_ap=tab3[:, 1, :],
                            idxs_ap=i16_sb[:, (t0 + te) * 8:(t0 + te) * 8 + no // 16],
                            num_idxs=no, num_idxs_reg=no,
                            elem_size=ROWB, elem_step=2 * ROWB,
                            single_packet=False)
                    # a_dst expanded to edges: [128e, tw*H]
                    adps = adp.tile([P, tmax * HEADS], F32, tag="adps")
                    for t in range(tw):
                        nc.tensor.matmul(out=adps[:, t * H:(t + 1) * H],
                                         lhsT=ohd[:size, t, :], rhs=ad[:size, :H],
                                         start=True, stop=True)
                    # logits -> exp(leaky_relu) on ScalarE
                    lg = wpool.tile([P, tmax * HEADS], F32, tag="lg")
                    lg3 = lg[:, :tw * H].rearrange("p (t h) -> p t h", t=tw)
                    ad3 = adps[:, :tw * H].rearrange("p (t h) -> p t h", t=tw)
                    nc.vector.tensor_tensor(out=lg3, in0=g[:, :tw, 128:128 + H],
                                            in1=ad3, op=OP.add)
                    lk = wpool.tile([P, tmax * HEADS], F32, tag="lk")
                    nc.vector.tensor_scalar_mul(lk[:, :tw * H], lg[:, :tw * H], NEG_SLOPE)
                    nc.vector.tensor_tensor(out=lk[:, :tw * H], in0=lg[:, :tw * H],
                                            in1=lk[:, :tw * H], op=OP.max)
                    rhs = gpool.tile([P, tmax, RH], DT, tag="rhs")
                    nc.scalar.activation(out=rhs[:, :tw, 0:H],
                                         in_=lk[:, :tw * H].rearrange("p (t h) -> p t h", t=tw),
                                         func=AF.Exp)
                    # rhs features = exp * feat(fp8)
                    nc.vector.tensor_tensor(
                        out=rhs[:, :tw, H:RH].rearrange("p t (h c) -> p t h c", h=H),
                        in0=g[:, :tw, :].bitcast(F8)[:, :, 0:256].rearrange(
                            "p t (h c) -> p t h c", h=H),
                        in1=rhs[:, :tw, 0:H].to_broadcast([P, tw, H, out_c // H]),
                        op=OP.mult)
                    # aggregate into [win, RH]
                    ag = aggp.tile([win, RH], F32, tag="ag")
                    for t in range(tw):
                        nc.tensor.matmul(out=ag[:, :], lhsT=ohe[:, t, :], rhs=rhs[:, t, :],
                                         start=(t == 0), stop=(t == tw - 1))
                    # normalize (bias pre-folded into table feats) + relu
                    s = wpool.tile([win, HEADS], F32, tag="s")
                    nc.vector.tensor_scalar_max(s[:size, :H], ag[:size, 0:H], 1e-30)
                    nc.vector.reciprocal(out=s[:size, :H], in_=s[:size, :H])
                    ro = wpool.tile([win, out_c], DT, tag="ro")
                    if H == 1:
                        nc.scalar.activation(out=ro[:size, :], in_=ag[:size, 1:RH],
                                             func=AF.Relu, scale=s[:size, 0:1])
                    else:
                        on = wpool.tile([win, out_c], F32, tag="on")
                        nc.vector.tensor_tensor(
                            out=on[:size, :].rearrange("d (h c) -> d h c", h=H),
                            in0=ag[:size, H:RH].rearrange("d (h c) -> d h c", h=H),
                            in1=s[:size, :H].to_broadcast([size, H, out_c // H]), op=OP.mult)
                        nc.scalar.activation(out=ro[:size, :], in_=on[:size, :], func=AF.Relu)
                    if after_window is not None:
                        after_window(w, ro, size)
                    if pool_into is not None:
                        gm = wpool.tile([P, G], DT, tag="gm")
                        nc.sync.dma_start(out=gm[:, :], in_=gmap_d[w, :, :])
                        for mh in range(2):
                            nc.tensor.matmul(out=pool_into[mh][:, :],
                                             lhsT=ro[:size, mh * P:(mh + 1) * P],
                                             rhs=gm[:size, :],
                                             start=(w == 0), stop=(w == nwin - 1))

            def l1_after_window(w, ro, size):
                def l2_lhsT(t, kh, rows):
                    tp = trp.tile([P, P], DT, tag="tp", name="tp")
                    nc.tensor.transpose(out=tp[:, :rows], in_=ro[:rows, kh * P:(kh + 1) * P],
                                        identity=ident[:rows, :rows])
                    tl = wpool.tile([P, P], DT, tag="tl", name="tl")
                    nc.vector.tensor_copy(out=tl[:, :rows], in_=tp[:, :rows])
                    return tl[:, :rows]
                node_tile(w, l2_lhsT, w2_sb, 258, cin2, b2_sb)
                if w in chunk_end_tile:
                    ag_chunk(cin2, tab2, chunk_end_tile[w])

            # pre-zero rotating gather buffers: trailing slots beyond each
            # gather's num_idxs are never written and must not hold NaN bits
            for _ in range(3):
                gz = gpool.tile([P, tmax, ROWB], DT, tag="g")
                nc.vector.memset(gz[:, :, :], 0)

            edge_phase(tab1_3, cin1, HEADS, after_window=l1_after_window)

            assert win == P and nwin == nt
            pps = [plp.tile([P, G], F32, tag=f"pp{mh}", name=f"pp{mh}") for mh in range(2)]
            edge_phase(tab2_3, cin2, 1, pool_into=pps)

            # ---- pool + fc ----
            psb = wpool.tile([P, 2, G], F32, tag="psb")
            for mh in range(2):
                nc.vector.tensor_copy(out=psb[:, mh, :], in_=pps[mh][:, :])
            nc.sync.dma_start(out=pin.ap().rearrange("(mh p) g -> p mh g", p=P), in_=psb[:, :, :])

            nc.gpsimd.collective_compute(
                "AllReduce", mybir.AluOpType.add,
                ins=[pin.ap()], outs=[pout.ap()], replica_groups=groups)

            pr = wpool.tile([P, 2, G], F32, tag="pr")
            nc.sync.dma_start(out=pr[:, :, :], in_=pout.ap().rearrange("(mh p) g -> p mh g", p=P))
            pm = wpool.tile([P, 2, G], DT, tag="pm")
            for kh in range(2):
                nc.vector.tensor_tensor(out=pm[:, kh, :], in0=pr[:, kh, :], in1=cinv_sb[:, :], op=OP.mult)
            for mh in range(2):
                fps = aggp.tile([P, G], F32, tag="ag")
                for kh in range(2):
                    nc.tensor.matmul(out=fps[:, :], lhsT=wfc_sb[:, kh, mh, :], rhs=pm[:, kh, :],
                                     start=(kh == 0), stop=(kh == 1))
                yo = wpool.tile([P, G], F32, tag="yo")
                nc.scalar.activation(out=yo[:, :], in_=fps[:, :], func=AF.Relu,
                                     bias=bfc_sb[:, mh:mh + 1], scale=1.0)
                nc.sync.dma_start(out=y_d[mh * P:(mh + 1) * P, :], in_=yo[:, :])

    nc.compile()
    return nc




def _install_ntff_hook():
    """Register the NTFF profile hook (the image's antenv lacks axon_hooks)."""
    import types
    mod = sys.modules.get("antenv.axon_hooks")
    if mod is None:
        import antenv
        mod = types.ModuleType("antenv.axon_hooks")
        mod._hook = None
        mod.set_axon_ntff_profile_hook = lambda h: setattr(mod, "_hook", h)
        mod.get_axon_ntff_profile_hook = lambda: mod._hook
        sys.modules["antenv.axon_hooks"] = mod
        antenv.axon_hooks = mod
    if mod._hook is None:
        from trn_agent_boot.trn_boot import _ntff_profile_via_ctypes
        mod.set_axon_ntff_profile_hook(_ntff_profile_via_ctypes("/opt/axon/libaxon_pjrt.so"))

# --------------------------------------------------------------------------
# entry point
# --------------------------------------------------------------------------

def kernel(**inputs) -> np.ndarray:
    global LAST_EXEC_NS
    from concourse.bass_utils import run_bass_kernel_spmd

    args = {k: np.asarray(v) for k, v in inputs.items()}
    perm = balance_nodes(args["edge_index"][1], N_NODES, NCORES, WIN)
    old_of_new = np.argsort(perm)
    args["x"] = args["x"][old_of_new]
    args["batch"] = args["batch"][old_of_new]
    ei = args["edge_index"]
    args["edge_index"] = np.stack([perm[ei[0]], perm[ei[1]]]).astype(ei.dtype)
    TE, TO, NE, NO, woff, has_b1, has_b2, per_core = build_host_inputs(
        args["x"], args["edge_index"], args["batch"],
        args["W1"], args["att_src1"], args["att_dst1"], args["b1"],
        args["W2"], args["att_src2"], args["att_dst2"], args["b2"],
        args["Wfc"], args["bfc"],
        N_NODES, N_GRAPHS, NCORES, WIN)
    nc = build_program(TE, TO, NE, NO, woff, has_b1, has_b2, N_NODES, N_GRAPHS, NCORES, WIN)

    trace = os.environ.get("GAT_TRACE") == "1"
    if trace:
        try:
            _install_ntff_hook()
        except Exception:
            trace = False
    res = run_bass_kernel_spmd(nc, per_core, core_ids=list(range(NCORES)), trace=trace)
    LAST_EXEC_NS = res.exec_time_ns
    y = res.results[0]["y"]
    return np.ascontiguousarray(y.T).astype(np.float32)


# revision 29
# speedup vs baseline: 1.0175x; 1.0175x over previous
"""GAT (2-layer, PyG-style) on 8 Trainium2 NeuronCores.

Strategy (node/graph-parallel, dst-sharded edges):
  - Nodes partitioned into 8 contiguous ranges (6250/core); edges assigned to
    the core owning their DST node, sorted by dst, grouped per 128-dst window
    and split by table-row parity (see below), padded to 128-edge tiles.
  - Node phase sharded: each core computes its rows of the gather table
    (512-B rows: 256 fp8 features [+bias], then bf16 a_src/a_dst coefs),
    then a chunked AllGather replicates the table to every core's DRAM.
  - Edge phase per core: per-edge source rows fetched with TWO batched
    dma_gather instructions per dst-window (the int16-index SWDGE gather;
    parity split with a 1024-B stride makes 50000 rows addressable with
    int16 indices).  Attention logits assembled on-chip; segment softmax +
    scatter-add are TensorE matmuls against 0/1 onehot matrices uploaded as
    u8 (shared by both layers) and cast on-chip; exp folded into the matmul
    rhs, 1/sum applied per-dst at the end.  leaky_relu/exp run on ScalarE.
  - Global mean-pool via matmul with a host-built node->graph 0/1 map,
    AllReduce of the [256, 50] partial, then the FC layer replicated.
"""

import os
import sys

sys.path.insert(0, "/opt/trn_rl_repo")

import numpy as np
import ml_dtypes

N_NODES, N_EDGES = 50000, 800000
IN_C, HID_C, OUT_C, HEADS = 256, 64, 256, 4
N_GRAPHS = 50
NEG_SLOPE = 0.2
NCORES = 8
WIN = 128         # dst nodes per aggregation window (psum partitions)
ROWB = 256        # table row width in bf16 elems (512 bytes)
P = 128

BF16 = ml_dtypes.bfloat16

LAST_EXEC_NS = None  # set by kernel() when GAT_TRACE=1


# --------------------------------------------------------------------------
# host-side preprocessing
# --------------------------------------------------------------------------

def build_edge_data(src, dst, rowmap, n_nodes, ncores, win):
    """Per-core int16 gather indices + u8 onehot matrices.

    Edges are dst-sorted, grouped per (core, window), split by parity of
    their source's table row (dma_gather int16 indices address rows at a
    1024-B stride: even rows via base 0, odd rows via base +512 B), and
    padded to 128-edge tiles per parity.

    Returns (TE, TO, woff, percore): TE[w]/TO[w] = even/odd tile count of
    window w (same for all cores); percore[c] = dict(
      i16 [128, Ttot*8] i16, ohe [128, Ttot, 128] u8, ohd [128, Ttot, 128] u8).
    """
    nc_nodes = n_nodes // ncores
    nwin = (nc_nodes + win - 1) // win
    E = src.shape[0]
    r = rowmap[src]                      # table row of each edge's source
    par = (r & 1).astype(np.int64)
    ridx = (r >> 1).astype(np.int16)

    core_of = dst // nc_nodes
    win_of = (dst % nc_nodes) // win
    dloc = (dst - (core_of * nc_nodes + win_of * win)).astype(np.int64)

    # group edges by (core, window, parity) — groups must be contiguous for
    # the rank-in-group slot assignment below
    order = np.lexsort((dst, par, win_of, core_of))
    s_core, s_win, s_par = core_of[order], win_of[order], par[order]
    s_ridx, s_dloc = ridx[order], dloc[order]

    gid = (s_core * nwin + s_win) * 2 + s_par
    sizes = np.bincount(gid, minlength=ncores * nwin * 2)
    starts = np.concatenate([[0], np.cumsum(sizes)])[:-1]
    k_in_g = np.arange(E) - starts[gid]

    cnt = sizes.reshape(ncores, nwin, 2)
    TE = np.maximum((cnt[:, :, 0].max(axis=0) + P - 1) // P, 1).astype(np.int64)
    TO = np.maximum((cnt[:, :, 1].max(axis=0) + P - 1) // P, 1).astype(np.int64)
    # gather index counts: max real count across cores rounded to 16 (the
    # output still covers TE/TO full tiles; trailing slots are never written
    # and rely on pre-zeroed gather buffers)
    NE = np.clip(((cnt[:, :, 0].max(axis=0) + 15) // 16) * 16, 16, TE * P).astype(np.int64)
    NO = np.clip(((cnt[:, :, 1].max(axis=0) + 15) // 16) * 16, 16, TO * P).astype(np.int64)
    twin = TE + TO
    woff = np.concatenate([[0], np.cumsum(twin)])[:-1]
    ttot = int(twin.sum())

    base_tile = woff[s_win] + np.where(s_par == 1, TE[s_win], 0)
    gtile = base_tile + k_in_g // P
    lane = (k_in_g % P).astype(np.int64)
    # int16 index array position: block col base + k//16, row k%16
    col = base_tile * 8 + k_in_g // 16
    row16 = (k_in_g % 16).astype(np.int64)

    percore = []
    for c in range(ncores):
        m = s_core == c
        i16 = np.zeros((16, ttot * 8), dtype=np.int16)
        i16[row16[m], col[m]] = s_ridx[m]
        ohe = np.zeros((P, ttot, P), dtype=np.uint8)
        ohd = np.zeros((P, ttot, P), dtype=np.uint8)
        ohe[lane[m], gtile[m], s_dloc[m]] = 1
        ohd[s_dloc[m], gtile[m], lane[m]] = 1
        percore.append(dict(i16=np.tile(i16, (8, 1)), ohe=ohe, ohd=ohd))
    return TE, TO, NE, NO, woff, percore


def balance_nodes(dst, n_nodes, ncores, win):
    """Relabel nodes so each (core, window) bin carries a near-equal edge
    count: perm[old_id] = new_id. Greedy largest-degree-first into the
    lightest non-full bin."""
    import heapq
    deg = np.bincount(dst, minlength=n_nodes).astype(np.int64)
    nc_nodes = n_nodes // ncores
    nwin = (nc_nodes + win - 1) // win
    base = []
    cap = []
    for c in range(ncores):
        for w in range(nwin):
            base.append(c * nc_nodes + w * win)
            cap.append(min(win, nc_nodes - w * win))
    nbins = len(base)
    order = np.argsort(-deg, kind="stable")
    heap = [(0, b) for b in range(nbins)]
    heapq.heapify(heap)
    slot = [0] * nbins
    perm = np.zeros(n_nodes, dtype=np.int64)
    for node in order:
        while True:
            load, b = heapq.heappop(heap)
            if slot[b] < cap[b]:
                break
        perm[node] = base[b] + slot[b]
        slot[b] += 1
        if slot[b] < cap[b]:
            heapq.heappush(heap, (load + deg[node], b))
    return perm

CHUNK_TILES = 13  # node-tiles per AllGather chunk


def chunk_layout(n_nodes, ncores, chunk_tiles):
    """Chunked-AllGather table layout. Returns (bounds, rowmap) where bounds
    are per-core local row boundaries of each chunk and rowmap[node] is the
    table row of a global node id under chunk-major ordering."""
    nc_nodes = n_nodes // ncores
    bounds = []
    lo = 0
    while lo < nc_nodes:
        hi = min(lo + chunk_tiles * P, nc_nodes)
        bounds.append((lo, hi))
        lo = hi
    rowmap = np.zeros(n_nodes, dtype=np.int64)
    out_base = 0
    for (lo, hi) in bounds:
        s = hi - lo
        for c in range(ncores):
            nodes = np.arange(c * nc_nodes + lo, c * nc_nodes + hi)
            rowmap[nodes] = out_base + c * s + np.arange(s)
        out_base += ncores * s
    return bounds, rowmap


def build_host_inputs(x, edge_index, batch, W1, att_src1, att_dst1, b1,
                      W2, att_src2, att_dst2, b2, Wfc, bfc,
                      n_nodes, n_graphs, ncores, win):
    src, dst = np.asarray(edge_index[0]), np.asarray(edge_index[1])
    nc_nodes = n_nodes // ncores
    nt = (nc_nodes + P - 1) // P
    ncpad = nt * P

    bounds, rowmap = chunk_layout(n_nodes, ncores, CHUNK_TILES)
    TE, TO, NE, NO, woff, edata = build_edge_data(
        src.astype(np.int64), dst.astype(np.int64), rowmap,
        n_nodes, ncores, win)

    # augmented weights: a = x @ (W @ att) computed in the same matmul as h
    in_c = W1.shape[0]
    A1 = np.zeros((in_c, 2 * HEADS), dtype=np.float64)
    for h in range(HEADS):
        A1[:, h] = W1[:, h * HID_C:(h + 1) * HID_C].astype(np.float64) @ att_src1[h].astype(np.float64)
        A1[:, HEADS + h] = W1[:, h * HID_C:(h + 1) * HID_C].astype(np.float64) @ att_dst1[h].astype(np.float64)
    W1aug = np.concatenate([W1.astype(np.float64), A1], axis=1).astype(BF16)  # [in_c, 264]

    hid2 = W2.shape[0]
    A2 = np.zeros((hid2, 2), dtype=np.float64)
    A2[:, 0] = W2.astype(np.float64) @ att_src2[0].astype(np.float64)
    A2[:, 1] = W2.astype(np.float64) @ att_dst2[0].astype(np.float64)
    W2aug = np.concatenate([W2.astype(np.float64), A2], axis=1).astype(BF16)  # [hid2, 258]

    # graph-mean map and counts
    cnt = np.bincount(batch, minlength=n_graphs).astype(np.float32)
    cnt_inv = (1.0 / np.maximum(cnt, 1.0)).astype(np.float32)

    out_c = Wfc.shape[0]
    has_b1 = bool(np.any(b1))
    has_b2 = bool(np.any(b2))
    common = dict(
        w1aug=np.ascontiguousarray(W1aug),
        w2aug=np.ascontiguousarray(W2aug),
        wfc=np.ascontiguousarray(Wfc.astype(BF16)),
        bfc2=np.ascontiguousarray(bfc.astype(np.float32).reshape(2, P).T.copy()),
        cinv=np.ascontiguousarray(np.broadcast_to(cnt_inv, (P, n_graphs)).copy()),
    )
    if has_b1:
        common["b1rep"] = np.ascontiguousarray(
            np.broadcast_to(b1.astype(np.float32), (P, b1.shape[0])).copy())
    if has_b2:
        common["b2rep"] = np.ascontiguousarray(
            np.broadcast_to(b2.astype(np.float32), (P, b2.shape[0])).copy())

    per_core = []
    for c in range(ncores):
        xt = np.zeros((in_c, ncpad), dtype=BF16)
        xs = x[c * nc_nodes:(c + 1) * nc_nodes].astype(np.float32)
        xt[:, :nc_nodes] = np.ascontiguousarray(xs.T).astype(BF16)
        gmap = np.zeros((nt, P, n_graphs), dtype=np.float32)
        nodes = np.arange(nc_nodes)
        gmap[nodes // P, nodes % P, batch[c * nc_nodes:(c + 1) * nc_nodes]] = 1.0
        d = edata[c]
        per_core.append(dict(
            xt=xt,
            i16=np.ascontiguousarray(d["i16"]),
            ohe=np.ascontiguousarray(d["ohe"]),
            ohd=np.ascontiguousarray(d["ohd"]),
            gmap=np.ascontiguousarray(gmap.astype(BF16)),
            **common,
        ))
    return TE, TO, NE, NO, woff, has_b1, has_b2, per_core


# --------------------------------------------------------------------------
# device program
# --------------------------------------------------------------------------

def build_program(TE, TO, NE, NO, woff, has_b1, has_b2, n_nodes, n_graphs, ncores, win,
                  dma_scratch=49152):
    bounds, _ = chunk_layout(n_nodes, ncores, CHUNK_TILES)
    from concourse import bass, bacc, mybir, tile
    from concourse.masks import make_identity

    DT = mybir.dt.bfloat16
    F8 = mybir.dt.float8e4
    U8 = mybir.dt.uint8
    F32 = mybir.dt.float32
    AF = mybir.ActivationFunctionType
    OP = mybir.AluOpType

    nc_nodes = n_nodes // ncores
    nt = (nc_nodes + P - 1) // P
    nwin = (nc_nodes + win - 1) // win
    twin = TE + TO
    ttot = int(np.sum(twin))
    tmax = int(np.max(twin))
    in_c, out_c = IN_C, OUT_C
    G = n_graphs

    nc = bacc.Bacc("TRN2", target_bir_lowering=False, num_devices=ncores,
                   dynamic_dma_scratch_size=dma_scratch)

    # ---- dram i/o ----
    xt_d = nc.dram_tensor("xt", [in_c, nt * P], DT, kind="ExternalInput")
    w1_d = nc.dram_tensor("w1aug", [in_c, 264], DT, kind="ExternalInput")
    w2_d = nc.dram_tensor("w2aug", [in_c, 258], DT, kind="ExternalInput")
    wfc_d = nc.dram_tensor("wfc", [out_c, out_c], DT, kind="ExternalInput")
    i16_d = nc.dram_tensor("i16", [P, ttot * 8], mybir.dt.int16, kind="ExternalInput")
    ohe_d = nc.dram_tensor("ohe", [P, ttot, P], U8, kind="ExternalInput")
    ohd_d = nc.dram_tensor("ohd", [P, ttot, P], U8, kind="ExternalInput")
    gmap_d = nc.dram_tensor("gmap", [nt, P, G], DT, kind="ExternalInput")
    bfc_d = nc.dram_tensor("bfc2", [P, 2], F32, kind="ExternalInput")
    cinv_d = nc.dram_tensor("cinv", [P, G], F32, kind="ExternalInput")
    b1_d = nc.dram_tensor("b1rep", [P, out_c], F32, kind="ExternalInput") if has_b1 else None
    b2_d = nc.dram_tensor("b2rep", [P, out_c], F32, kind="ExternalInput") if has_b2 else None
    y_d = nc.dram_tensor("y", [out_c, G], F32, kind="ExternalOutput")

    cin1 = nc.dram_tensor("cin1", [nc_nodes, ROWB], DT, kind="Internal")
    tab1 = nc.dram_tensor("tab1", [n_nodes, ROWB], DT, kind="Internal", addr_space="Shared")
    cin2 = nc.dram_tensor("cin2", [nc_nodes, ROWB], DT, kind="Internal")
    tab2 = nc.dram_tensor("tab2", [n_nodes, ROWB], DT, kind="Internal", addr_space="Shared")
    pin = nc.dram_tensor("pin", [out_c, G], F32, kind="Internal")
    pout = nc.dram_tensor("pout", [out_c, G], F32, kind="Internal", addr_space="Shared")

    groups = [list(range(ncores))]

    with tile.TileContext(nc) as tc:
        with (
            tc.tile_pool(name="const", bufs=1) as cpool,
            tc.tile_pool(name="work", bufs=3) as wpool,
            tc.tile_pool(name="gath", bufs=3) as gpool,
            tc.tile_pool(name="np", bufs=2, space="PSUM") as npp,
            tc.tile_pool(name="agg", bufs=2, space="PSUM") as aggp,
            tc.tile_pool(name="adp", bufs=1, space="PSUM") as adp,
            tc.tile_pool(name="trp", bufs=1, space="PSUM") as trp,
            tc.tile_pool(name="plp", bufs=1, space="PSUM") as plp,
        ):
            # ---- constants ----
            ident = cpool.tile([P, P], DT)
            make_identity(nc, ident[:])
            w1_sb = cpool.tile([P, 2, 264], DT)
            nc.sync.dma_start(out=w1_sb[:, :, :], in_=w1_d.ap().rearrange("(kh p) m -> p kh m", p=P))
            w2_sb = cpool.tile([P, 2, 258], DT)
            nc.sync.dma_start(out=w2_sb[:, :, :], in_=w2_d.ap().rearrange("(kh p) m -> p kh m", p=P))
            wfc_sb = cpool.tile([P, 2, 2, P], DT)  # [k-half, m-half]
            nc.sync.dma_start(out=wfc_sb[:, :, :, :],
                              in_=wfc_d.ap().rearrange("(kh p) (mh q) -> p kh mh q", p=P, q=P))
            bfc_sb = cpool.tile([P, 2], F32)
            nc.sync.dma_start(out=bfc_sb[:, :], in_=bfc_d[:, :])
            cinv_sb = cpool.tile([P, G], F32)
            nc.sync.dma_start(out=cinv_sb[:, :], in_=cinv_d[:, :])
            i16_sb = cpool.tile([P, ttot * 8], mybir.dt.int16)
            nc.sync.dma_start(out=i16_sb[:, :], in_=i16_d[:, :])
            b1_sb = b2_sb = None
            if has_b1:
                b1_sb = cpool.tile([P, out_c], F32)
                nc.sync.dma_start(out=b1_sb[:, :], in_=b1_d[:, :])
            if has_b2:
                b2_sb = cpool.tile([P, out_c], F32)
                nc.sync.dma_start(out=b2_sb[:, :], in_=b2_d[:, :])

            tab1_3 = tab1.ap().rearrange("(r two) c -> r two c", two=2)
            tab2_3 = tab2.ap().rearrange("(r two) c -> r two c", two=2)

            # ---- node phase ----
            chunk_end_tile = {}
            chunk_ob = []
            ob = 0
            for k, (lo, hi) in enumerate(bounds):
                chunk_end_tile[(hi + P - 1) // P - 1] = k
                chunk_ob.append(ob)
                ob += ncores * (hi - lo)

            def ag_chunk(cin, tab, k):
                lo, hi = bounds[k]
                s = hi - lo
                nc.gpsimd.collective_compute(
                    "AllGather", mybir.AluOpType.bypass,
                    ins=[cin.ap()[lo:hi, :]],
                    outs=[tab.ap()[chunk_ob[k]:chunk_ob[k] + ncores * s, :]],
                    replica_groups=groups)

            def node_tile(t, lhsT_of, w_sb, ocols, cin, brep):
                rows = min(P, nc_nodes - t * P)
                na = ocols - 256
                ps = npp.tile([P, 264], F32, tag="nps", name="nps")
                for kh in range(2):
                    nc.tensor.matmul(out=ps[:rows, :ocols], lhsT=lhsT_of(t, kh, rows),
                                     rhs=w_sb[:, kh, :ocols], start=(kh == 0), stop=(kh == 1))
                sb = wpool.tile([P, ROWB], DT, tag="nsb", name="nsb")
                f8v = sb[:rows, :].bitcast(F8)
                if brep is not None:
                    nc.vector.tensor_tensor(out=f8v[:, 0:256], in0=ps[:rows, 0:256],
                                            in1=brep[:rows, :], op=OP.add)
                else:
                    nc.vector.tensor_copy(out=f8v[:, 0:256], in_=ps[:rows, 0:256])
                nc.vector.tensor_copy(out=sb[:rows, 128:128 + na], in_=ps[:rows, 256:ocols])
                nc.vector.memset(sb[:rows, 128 + na:ROWB], 0)
                nc.sync.dma_start(out=cin.ap()[t * P:t * P + rows, :], in_=sb[:rows, :])

            def node_phase(lhsT_of, w_sb, ocols, cin, brep, tab=None):
                for t in range(nt):
                    node_tile(t, lhsT_of, w_sb, ocols, cin, brep)
                    if tab is not None and t in chunk_end_tile:
                        ag_chunk(cin, tab, chunk_end_tile[t])

            # layer-1 node phase: xT staged in bulk DMAs, sliced per tile
            xt_sb = cpool.tile([P, 2, nt * P], DT)
            xstep = ((nt + 4) // 5) * P
            for lo in range(0, nt * P, xstep):
                hi = min(lo + xstep, nt * P)
                for kh in range(2):
                    nc.sync.dma_start(out=xt_sb[:, kh, lo:hi], in_=xt_d[kh * P:(kh + 1) * P, lo:hi])
            def l1_lhsT(t, kh, rows):
                return xt_sb[:, kh, t * P:t * P + rows]
            node_phase(l1_lhsT, w1_sb, 264, cin1, b1_sb, tab=tab1)

            # ---- edge phase ----
            def edge_phase(tab3, cin, H, after_window=None, pool_into=None):
                """Table row bf16-col layout: feats fp8 in bytes [0:256)
                (bf16 cols 0:128), a_src bf16 cols 128:128+H, a_dst cols
                128+H:128+2H."""
                RH = H + out_c
                for w in range(nwin):
                    size = min(win, nc_nodes - w * win)
                    te, to = int(TE[w]), int(TO[w])
                    tw = te + to
                    t0 = int(woff[w])
                    # own-dst attention coefs
                    ad = wpool.tile([win, HEADS], DT, tag="adst")
                    nc.sync.dma_start(out=ad[:size, :H],
                                      in_=cin.ap()[w * win:w * win + size, 128 + H:128 + 2 * H])
                    # onehots: u8 upload (shared by both layers), cast on-chip
                    oheu = wpool.tile([P, tmax, P], U8, tag="oheu")
                    nc.sync.dma_start(out=oheu[:, :tw, :], in_=ohe_d.ap()[:, t0:t0 + tw, :])
                    ohe = wpool.tile([P, tmax, win], DT, tag="ohe")
                    nc.vector.tensor_copy(out=ohe[:, :tw, :], in_=oheu[:, :tw, :])
                    ohdu = wpool.tile([win, tmax, P], U8, tag="ohdu")
                    nc.sync.dma_start(out=ohdu[:, :tw, :], in_=ohd_d.ap()[:, t0:t0 + tw, :])
                    ohd = wpool.tile([win, tmax, P], DT, tag="ohd")
                    nc.vector.tensor_copy(out=ohd[:, :tw, :], in_=ohdu[:, :tw, :])
                    # gather source rows: one batched SWDGE gather per parity
                    g = gpool.tile([P, tmax, ROWB], DT, tag="g")
                    ne, no = int(NE[w]), int(NO[w])
                    if te:
                        nc.gpsimd.dma_gather(
                            out_ap=g[:, 0:te, :], in_ap=tab3[:, 0, :],
                            idxs_ap=i16_sb[:, t0 * 8:t0 * 8 + ne // 16],
                            num_idxs=ne, num_idxs_reg=ne,
                            elem_size=ROWB, elem_step=2 * ROWB,
                            single_packet=False)
                    if to:
                        nc.gpsimd.dma_gather(
                            out_ap=g[:, te:tw, :], in_ap=tab3[:, 1, :],
                            idxs_ap=i16_sb[:, (t0 + te) * 8:(t0 + te) * 8 + no // 16],
                            num_idxs=no, num_idxs_reg=no,
                            elem_size=ROWB, elem_step=2 * ROWB,
                            single_packet=False)
                    # a_dst expanded to edges: [128e, tw*H]
                    adps = adp.tile([P, tmax * HEADS], F32, tag="adps")
                    for t in range(tw):
                        nc.tensor.matmul(out=adps[:, t * H:(t + 1) * H],
                                         lhsT=ohd[:size, t, :], rhs=ad[:size, :H],
                                         start=True, stop=True)
                    # logits -> exp(leaky_relu) on ScalarE
                    lg = wpool.tile([P, tmax * HEADS], F32, tag="lg")
                    lg3 = lg[:, :tw * H].rearrange("p (t h) -> p t h", t=tw)
                    ad3 = adps[:, :tw * H].rearrange("p (t h) -> p t h", t=tw)
                    nc.vector.tensor_tensor(out=lg3, in0=g[:, :tw, 128:128 + H],
                                            in1=ad3, op=OP.add)
                    lk = wpool.tile([P, tmax * HEADS], F32, tag="lk")
                    nc.vector.tensor_scalar_mul(lk[:, :tw * H], lg[:, :tw * H], NEG_SLOPE)
                    nc.vector.tensor_tensor(out=lk[:, :tw * H], in0=lg[:, :tw * H],
                                            in1=lk[:, :tw * H], op=OP.max)
                    rhs = gpool.tile([P, tmax, RH], DT, tag="rhs")
                    nc.scalar.activation(out=rhs[:, :tw, 0:H],
                                         in_=lk[:, :tw * H].rearrange("p (t h) -> p t h", t=tw),
                                         func=AF.Exp)
                    # rhs features = exp * feat(fp8)
                    nc.vector.tensor_tensor(
                        out=rhs[:, :tw, H:RH].rearrange("p t (h c) -> p t h c", h=H),
                        in0=g[:, :tw, :].bitcast(F8)[:, :, 0:256].rearrange(
                            "p t (h c) -> p t h c", h=H),
                        in1=rhs[:, :tw, 0:H].to_broadcast([P, tw, H, out_c // H]),
                        op=OP.mult)
                    # aggregate into [win, RH]
                    ag = aggp.tile([win, RH], F32, tag="ag")
                    for t in range(tw):
                        nc.tensor.matmul(out=ag[:, :], lhsT=ohe[:, t, :], rhs=rhs[:, t, :],
                                         start=(t == 0), stop=(t == tw - 1))
                    # normalize (bias pre-folded into table feats) + relu
                    s = wpool.tile([win, HEADS], F32, tag="s")
                    nc.vector.tensor_scalar_max(s[:size, :H], ag[:size, 0:H], 1e-30)
                    nc.vector.reciprocal(out=s[:size, :H], in_=s[:size, :H])
                    ro = wpool.tile([win, out_c], DT, tag="ro")
                    if H == 1:
                        nc.scalar.activation(out=ro[:size, :], in_=ag[:size, 1:RH],
                                             func=AF.Relu, scale=s[:size, 0:1])
                    else:
                        on = wpool.tile([win, out_c], F32, tag="on")
                        nc.vector.tensor_tensor(
                            out=on[:size, :].rearrange("d (h c) -> d h c", h=H),
                            in0=ag[:size, H:RH].rearrange("d (h c) -> d h c", h=H),
                            in1=s[:size, :H].to_broadcast([size, H, out_c // H]), op=OP.mult)
                        nc.scalar.activation(out=ro[:size, :], in_=on[:size, :], func=AF.Relu)
                    if after_window is not None:
                        after_window(w, ro, size)
                    if pool_into is not None:
                        gm = wpool.tile([P, G], DT, tag="gm")
                        nc.sync.dma_start(out=gm[:, :], in_=gmap_d[w, :, :])
                        for mh in range(2):
                            nc.tensor.matmul(out=pool_into[mh][:, :],
                                             lhsT=ro[:size, mh * P:(mh + 1) * P],
                                             rhs=gm[:size, :],
                                             start=(w == 0), stop=(w == nwin - 1))

            def l1_after_window(w, ro, size):
                def l2_lhsT(t, kh, rows):
                    tp = trp.tile([P, P], DT, tag="tp", name="tp")
                    nc.tensor.transpose(out=tp[:, :rows], in_=ro[:rows, kh * P:(kh + 1) * P],
                                        identity=ident[:rows, :rows])
                    tl = wpool.tile([P, P], DT, tag="tl", name="tl")
                    nc.vector.tensor_copy(out=tl[:, :rows], in_=tp[:, :rows])
                    return tl[:, :rows]
                node_tile(w, l2_lhsT, w2_sb, 258, cin2, b2_sb)
                if w in chunk_end_tile:
                    ag_chunk(cin2, tab2, chunk_end_tile[w])

            # pre-zero rotating gather buffers: trailing slots beyond each
            # gather's num_idxs are never written and must not hold NaN bits
            for _ in range(3):
                gz = gpool.tile([P, tmax, ROWB], DT, tag="g")
                nc.vector.memset(gz[:, :, :], 0)

            edge_phase(tab1_3, cin1, HEADS, after_window=l1_after_window)

            assert win == P and nwin == nt
            pps = [plp.tile([P, G], F32, tag=f"pp{mh}", name=f"pp{mh}") for mh in range(2)]
            edge_phase(tab2_3, cin2, 1, pool_into=pps)

            # ---- pool + fc ----
            psb = wpool.tile([P, 2, G], F32, tag="psb")
            for mh in range(2):
                nc.vector.tensor_copy(out=psb[:, mh, :], in_=pps[mh][:, :])
            nc.sync.dma_start(out=pin.ap().rearrange("(mh p) g -> p mh g", p=P), in_=psb[:, :, :])

            nc.gpsimd.collective_compute(
                "AllReduce", mybir.AluOpType.add,
                ins=[pin.ap()], outs=[pout.ap()], replica_groups=groups)

            pr = wpool.tile([P, 2, G], F32, tag="pr")
            nc.sync.dma_start(out=pr[:, :, :], in_=pout.ap().rearrange("(mh p) g -> p mh g", p=P))
            pm = wpool.tile([P, 2, G], DT, tag="pm")
            for kh in range(2):
                nc.vector.tensor_tensor(out=pm[:, kh, :], in0=pr[:, kh, :], in1=cinv_sb[:, :], op=OP.mult)
            for mh in range(2):
                fps = aggp.tile([P, G], F32, tag="ag")
                for kh in range(2):
                    nc.tensor.matmul(out=fps[:, :], lhsT=wfc_sb[:, kh, mh, :], rhs=pm[:, kh, :],
                                     start=(kh == 0), stop=(kh == 1))
                yo = wpool.tile([P, G], F32, tag="yo")
                nc.scalar.activation(out=yo[:, :], in_=fps[:, :], func=AF.Relu,
                                     bias=bfc_sb[:, mh:mh + 1], scale=1.0)
                nc.sync.dma_start(out=y_d[mh * P:(mh + 1) * P, :], in_=yo[:, :])

    nc.compile()
    return nc




def _install_ntff_hook():
    """Register the NTFF profile hook (the image's antenv lacks axon_hooks)."""
    import types
    mod = sys.modules.get("antenv.axon_hooks")
    if mod is None:
        import antenv
        mod = types.ModuleType("antenv.axon_hooks")
        mod._hook = None
        mod.set_axon_ntff_profile_hook = lambda h: setattr(mod, "_hook", h)
        mod.get_axon_ntff_profile_hook = lambda: mod._hook
        sys.modules["antenv.axon_hooks"] = mod
        antenv.axon_hooks = mod
    if mod._hook is None:
        from trn_agent_boot.trn_boot import _ntff_profile_via_ctypes
        mod.set_axon_ntff_profile_hook(_ntff_profile_via_ctypes("/opt/axon/libaxon_pjrt.so"))

# --------------------------------------------------------------------------
# entry point
# --------------------------------------------------------------------------

def kernel(**inputs) -> np.ndarray:
    global LAST_EXEC_NS
    from concourse.bass_utils import run_bass_kernel_spmd

    args = {k: np.asarray(v) for k, v in inputs.items()}
    perm = balance_nodes(args["edge_index"][1], N_NODES, NCORES, WIN)
    old_of_new = np.argsort(perm)
    args["x"] = args["x"][old_of_new]
    args["batch"] = args["batch"][old_of_new]
    ei = args["edge_index"]
    args["edge_index"] = np.stack([perm[ei[0]], perm[ei[1]]]).astype(ei.dtype)
    TE, TO, NE, NO, woff, has_b1, has_b2, per_core = build_host_inputs(
        args["x"], args["edge_index"], args["batch"],
        args["W1"], args["att_src1"], args["att_dst1"], args["b1"],
        args["W2"], args["att_src2"], args["att_dst2"], args["b2"],
        args["Wfc"], args["bfc"],
        N_NODES, N_GRAPHS, NCORES, WIN)
    nc = build_program(TE, TO, NE, NO, woff, has_b1, has_b2, N_NODES, N_GRAPHS, NCORES, WIN)

    trace = os.environ.get("GAT_TRACE") == "1"
    if trace:
        try:
            _install_ntff_hook()
        except Exception:
            trace = False
    res = run_bass_kernel_spmd(nc, per_core, core_ids=list(range(NCORES)), trace=trace)
    LAST_EXEC_NS = res.exec_time_ns
    y = res.results[0]["y"]
    return np.ascontiguousarray(y.T).astype(np.float32)
